# revision 1
# baseline (speedup 1.0000x reference)
"""Trainium2 Bass kernel for nn_ConvFilter (dense_cnn).

Math: tau = sigmoid((x[:,-1,:] @ W.T + b)/10 - 3) in (0, ~0.08];
norm = 1-tau (tau^128 underflows in fp32); out[b,t,f] =
norm * y[b,t+127,f] with y[s] = tau*y[s-1] + x[s]. tau^8 ~ 1e-9, so
the filter is effectively 8 taps and the recurrence can be decimated.

v5 dataflow (per 128-feature chunk, [f, t] layout, x host-split into
even/odd time halves so every on-device access is contiguous):

  PE:   z''[s] = (tau^2 norm)*x[2s] + (tau norm)*x[2s+1]  (diag MMs)
  DVE:  w''    = scan(tau^2, z'')      # = tau*norm*y[2s+1]
  ACT:  os[odd block]  = (1/tau)*w''[63:]         # = norm*y_odd
  DVE:  os[even block] = norm*x[2s] + w''[s-1]    # = norm*y_even (STT)

os stores the two phases as contiguous blocks [odd 961 | even 960];
the host re-interleaves (along with the [f,t]->[t,f] transpose it
already does). Per-core: DVE ~57us, ACT ~28us, PE ~20-45us, DMA ~46us.

Sharding: data-parallel over batch B=32 across 8 cores (4 samples
each); W, b replicated. Heavy traffic bf16 (gate 2e-2, measured ~3e-3).
"""
import numpy as np
import ml_dtypes

B, L, F, K = 32, 2048, 512, 128
LOUT = L - K + 1  # 1921
NCORES = 8
BC = B // NCORES  # 4 batch samples per core
NF = F // 128     # 4 feature chunks
LH = L // 2       # 1024 decimated length
NODD = 961        # odd-phase outputs (t = 127, 129, ..., 2047)
NEVEN = 960       # even-phase outputs (t = 128, 130, ..., 2046)
BF16 = ml_dtypes.bfloat16

_CACHE = {}


def _build():
    import concourse.bacc as bacc
    import concourse.tile as tile
    from concourse import masks, mybir

    F32 = mybir.dt.float32
    BF = mybir.dt.bfloat16
    MULT = mybir.AluOpType.mult
    ADD = mybir.AluOpType.add
    COPY = mybir.ActivationFunctionType.Copy
    SIG = mybir.ActivationFunctionType.Sigmoid

    nc = bacc.Bacc("TRN2", target_bir_lowering=False, debug=False,
                   num_devices=NCORES)
    # x host-prepped: [BC, F, L] bf16, per row [even-t half | odd-t half]
    x_in = nc.dram_tensor("x", [BC, F, L], BF, kind="ExternalInput")
    w_in = nc.dram_tensor("W", [F, F], F32, kind="ExternalInput")
    b_in = nc.dram_tensor("b", [128, NF], F32, kind="ExternalInput")
    f_in = nc.dram_tensor("feat", [128, NF * BC], F32, kind="ExternalInput")
    # out per chunk: [odd-phase 961 | even-phase 960]; host re-interleaves
    out_t = nc.dram_tensor("out", [BC, F, LOUT], BF, kind="ExternalOutput")

    with tile.TileContext(nc) as tc:
        with (
            tc.tile_pool(name="const", bufs=1) as const,
            tc.tile_pool(name="xt", bufs=3) as xt_pool,
            tc.tile_pool(name="w", bufs=4) as w_pool,
            tc.tile_pool(name="os", bufs=2) as os_pool,
            tc.tile_pool(name="pz", bufs=3, space="PSUM") as pz_pool,
            tc.tile_pool(name="pp", bufs=1, space="PSUM") as pp_pool,
        ):
            # ---- phase 0: tau / norm / per-(bi,c) diag weights ----
            wn = const.tile([128, NF * F], F32)
            wn3 = wn[:].rearrange("p (c f) -> p c f", f=F)
            nc.sync.dma_start(wn3, w_in[:, :].rearrange("(c p) f -> p c f", p=128))
            featn = const.tile([128, NF * BC], F32)
            feat3 = featn[:].rearrange("p (c b) -> p c b", b=BC)
            nc.sync.dma_start(featn[:], f_in[:, :])
            btile = const.tile([128, NF], F32)
            nc.sync.dma_start(btile[:], b_in[:, :])
            bias = const.tile([128, NF], F32)
            nc.vector.tensor_scalar(bias[:], btile[:], 0.1, -3.0, MULT, ADD)

            identb = const.tile([128, 128], BF)
            masks.make_identity(nc, identb[:])

            tau = const.tile([128, NF * BC], F32)
            tau3 = tau[:].rearrange("p (c b) -> p c b", b=BC)
            tau2 = const.tile([128, NF * BC], F32)
            tau23 = tau2[:].rearrange("p (c b) -> p c b", b=BC)
            norm = const.tile([128, NF * BC], F32)
            norm3 = norm[:].rearrange("p (c b) -> p c b", b=BC)
            tnorm = const.tile([128, NF * BC], F32)     # tau*norm
            tnorm3 = tnorm[:].rearrange("p (c b) -> p c b", b=BC)
            t2norm = const.tile([128, NF * BC], F32)    # tau^2*norm
            t2norm3 = t2norm[:].rearrange("p (c b) -> p c b", b=BC)
            itau = const.tile([128, NF * BC], F32)      # 1/tau
            itau3 = itau[:].rearrange("p (c b) -> p c b", b=BC)
            # diag(tau^2 norm), diag(tau norm) per (c, b), bf16
            dgA = const.tile([128, NF * BC * 128], BF)
            dgA3 = dgA[:].rearrange("p (i f) -> p i f", f=128)
            dgB = const.tile([128, NF * BC * 128], BF)
            dgB3 = dgB[:].rearrange("p (i f) -> p i f", f=128)
            # all 16 tau-matmuls into one PSUM tile (no round-robin stalls)
            lp = pp_pool.tile([128, NF * BC], F32, tag="pp")
            lp3 = lp[:].rearrange("p (c b) -> p c b", b=BC)
            for co in range(NF):
                for ci in range(NF):
                    nc.tensor.matmul(lp3[:, co, :],
                                     wn3[:, ci, 128 * co:128 * (co + 1)],
                                     feat3[:, ci, :],
                                     start=(ci == 0), stop=(ci == NF - 1))
            for co in range(NF):
                nc.scalar.activation(tau3[:, co, :], lp3[:, co, :], SIG,
                                     bias=bias[:, co:co + 1], scale=0.1)
            nc.vector.tensor_scalar(norm[:], tau[:], -1.0, 1.0, MULT, ADD)
            nc.vector.tensor_tensor(tau2[:], tau[:], tau[:], MULT)
            nc.vector.tensor_tensor(tnorm[:], tau[:], norm[:], MULT)
            nc.vector.tensor_tensor(t2norm[:], tau2[:], norm[:], MULT)
            nc.vector.reciprocal(itau[:], tau[:])

            # ---- main: software-pipelined over 16 (bi, c) tiles ----
            tiles = [(bi, c) for bi in range(BC) for c in range(NF)]
            xt3s = [None] * BC
            os3s = [None] * BC
            state = [None] * 16

            def stage_front(i):
                bi, c = tiles[i]
                if c == 0:
                    xt = xt_pool.tile([128, NF * L], BF, tag="xt")
                    xt3s[bi] = xt[:].rearrange("p (c l) -> p c l", l=L)
                    nc.sync.dma_start(
                        xt3s[bi], x_in[bi].rearrange("(c p) l -> p c l", p=128))
                    os_ = os_pool.tile([128, NF * LOUT], BF, tag="os")
                    os3s[bi] = os_[:].rearrange("p (c t) -> p c t", t=LOUT)
                ii = BC * c + bi
                nc.scalar.activation(dgA3[:, ii, :], identb[:], COPY,
                                     scale=t2norm3[:, c, bi:bi + 1])
                nc.scalar.activation(dgB3[:, ii, :], identb[:], COPY,
                                     scale=tnorm3[:, c, bi:bi + 1])
                xe = xt3s[bi][:, c, 0:LH]        # x[2s]
                xo = xt3s[bi][:, c, LH:L]        # x[2s+1]
                z = pz_pool.tile([128, LH], F32, tag="pz")
                for h in range(2):
                    hs = slice(512 * h, 512 * (h + 1))
                    nc.tensor.matmul(z[:, hs], dgA3[:, ii, :], xe[:, hs],
                                     start=True, stop=False)
                for h in range(2):
                    hs = slice(512 * h, 512 * (h + 1))
                    nc.tensor.matmul(z[:, hs], dgB3[:, ii, :], xo[:, hs],
                                     start=False, stop=True)
                w = w_pool.tile([128, LH], BF, tag="w")
                nc.vector.tensor_tensor_scan(
                    w[:], tau23[:, c, bi:bi + 1].broadcast_to([128, LH]),
                    z[:], 0.0, MULT, ADD)
                state[i] = w

            def stage_back(i):
                bi, c = tiles[i]
                w = state[i]
                oc = os3s[bi][:, c, :]
                xe = xt3s[bi][:, c, 0:LH]
                # odd t: norm*y_odd = (1/tau) * w''
                nc.scalar.activation(oc[0:128, 0:NODD], w[:, 63:LH], COPY,
                                     scale=itau3[:, c, bi:bi + 1])
                # even t: norm*y_even = norm*x[2s] + w''[s-1]
                nc.vector.scalar_tensor_tensor(
                    oc[0:128, NODD:LOUT], xe[:, 64:LH],
                    norm3[:, c, bi:bi + 1], w[:, 63:LH - 1], MULT, ADD)
                nc.scalar.dma_start(
                    out_t[bi, 128 * c:128 * (c + 1), :], os3s[bi][:, c, :])

            stage_front(0)
            for i in range(1, 16):
                stage_front(i)
                stage_back(i - 1)
            stage_back(15)
    nc.compile()
    return nc


def _get_nc():
    if "nc" not in _CACHE:
        _CACHE["nc"] = _build()
    return _CACHE["nc"]


def _prep_in_maps(x: np.ndarray, W: np.ndarray, b: np.ndarray):
    x = np.ascontiguousarray(x, dtype=np.float32)
    xT = x.transpose(0, 2, 1).astype(BF16)            # [B, F, L] bf16
    # deinterleave time: [even half | odd half] so device accesses are
    # contiguous
    xd = np.empty_like(xT)
    xd[:, :, 0:LH] = xT[:, :, 0::2]
    xd[:, :, LH:L] = xT[:, :, 1::2]
    WT = np.ascontiguousarray(W.T, dtype=np.float32)  # [fi, fo]
    feat = x[:, L - 1, :].astype(np.float32)          # [B, F]
    b = np.ascontiguousarray(b, dtype=np.float32)
    b_r = np.ascontiguousarray(b.reshape(NF, 128).T)
    return [
        {"x": xd[i * BC:(i + 1) * BC], "W": WT, "b": b_r,
         "feat": np.ascontiguousarray(
             feat[i * BC:(i + 1) * BC].reshape(BC, NF, 128)
             .transpose(2, 1, 0).reshape(128, NF * BC))}
        for i in range(NCORES)
    ]


def kernel(x: np.ndarray, W: np.ndarray, b: np.ndarray) -> np.ndarray:
    from concourse.bass_utils import run_bass_kernel_spmd

    nc = _get_nc()
    in_maps = _prep_in_maps(x, W, b)
    res = run_bass_kernel_spmd(nc, in_maps, list(range(NCORES)))
    dev = np.concatenate(
        [np.asarray(res.results[i]["out"]) for i in range(NCORES)],
        axis=0).astype(np.float32)                     # [B, F, LOUT]
    out = np.empty((B, LOUT, F), dtype=np.float32)
    out[:, 0::2, :] = dev[:, :, 0:NODD].transpose(0, 2, 1)
    out[:, 1::2, :] = dev[:, :, NODD:LOUT].transpose(0, 2, 1)
    return out


if __name__ == "__main__":
    rng = np.random.default_rng(0)
    x = rng.standard_normal((B, L, F), dtype=np.float32)
    W = (rng.standard_normal((F, F), dtype=np.float32) / np.sqrt(F)).astype(np.float32)
    b = np.zeros((F,), dtype=np.float32)
    out = kernel(x, W, b)
    print("out", out.shape, out.dtype)

